# revision 4
# baseline (speedup 1.0000x reference)
"""ChebNet (3x ChebConv S=5 + global mean pool + 2-layer MLP) on 8 trn2 cores.

Strategy (graph-level data parallelism):
  - 64 independent graphs (1024 nodes each, edges strictly intra-graph).
    Core c owns graphs [8c, 8c+8).
  - Host prep: build per-graph dense scaled Laplacian  M2[src, dst] =
    2*Lhat[dst, src]  (edge weights + diagonal folded in, duplicate edges
    accumulated), transpose x per graph to feature-layout, concat the 5
    Chebyshev weight matrices per layer, fold the mean-pool 1/1024 into fcW1.
  - Device per graph per layer: Clenshaw recurrence
        b_k = 2L b_{k+1} - b_{k+2} + X W_k
    so every Lhat application is a dense [128,1024]^T @ [128,F] PSUM-chain
    matmul in node-layout; projections X W_k run from a feature-layout copy
    of the layer input (one PE-transpose per layer boundary). Pooling via
    matmul with a ones-vector; biases via K=1 ones-row matmuls.
"""

import os

import numpy as np

N_NODES = 65536
N_EDGES = 1048576
G = 64
NPG = 1024
IN_F = 128
HID = 64
NCLS = 10
S = 5
NCORES = 8
GPC = G // NCORES  # graphs per core

# (F_in, F_out) per ChebConv layer
LAYERS = [(128, 32), (32, 64), (64, 64)]

LAST = None  # BassKernelResults of the most recent run (for test harness)
_CACHE = {}


def _build_bass():
    from contextlib import ExitStack

    import concourse.bass as bass  # noqa: F401
    import concourse.tile as tile
    from concourse import bacc, mybir
    from concourse.masks import make_identity

    f32 = mybir.dt.float32
    Act = mybir.ActivationFunctionType
    Alu = mybir.AluOpType

    nc = bacc.Bacc(
        "TRN2",
        target_bir_lowering=False,
        debug=False,
        enable_asserts=False,
        num_devices=NCORES,
    )

    lt_d = nc.dram_tensor("lt2", [GPC, 128, 8 * 1024], f32, kind="ExternalInput").ap()
    xt_d = nc.dram_tensor("xt", [GPC, 128, 1024], f32, kind="ExternalInput").ap()
    wall_d = [
        nc.dram_tensor(f"wall{i}", [fi, 5 * fo], f32, kind="ExternalInput").ap()
        for i, (fi, fo) in enumerate(LAYERS)
    ]
    brow_d = [
        nc.dram_tensor(f"brow{i}", [1, 5 * fo], f32, kind="ExternalInput").ap()
        for i, (fi, fo) in enumerate(LAYERS)
    ]
    fcw1_d = nc.dram_tensor("fcw1", [HID, NCLS], f32, kind="ExternalInput").ap()
    fcb1_d = nc.dram_tensor("fcb1", [1, NCLS], f32, kind="ExternalInput").ap()
    fcw2_d = nc.dram_tensor("fcw2", [NCLS, NCLS], f32, kind="ExternalInput").ap()
    fcb2_d = nc.dram_tensor("fcb2", [1, NCLS], f32, kind="ExternalInput").ap()
    out_d = nc.dram_tensor("out", [GPC, NCLS], f32, kind="ExternalOutput").ap()

    with tile.TileContext(nc) as tc, ExitStack() as ctx:
        consts = ctx.enter_context(tc.tile_pool(name="consts", bufs=1))
        ltp = ctx.enter_context(tc.tile_pool(name="lt", bufs=2))
        xtp = ctx.enter_context(tc.tile_pool(name="xtp", bufs=2))
        xfp = ctx.enter_context(tc.tile_pool(name="xfp", bufs=2))
        up = ctx.enter_context(tc.tile_pool(name="up", bufs=2))
        bp = ctx.enter_context(tc.tile_pool(name="bp", bufs=2))
        hp = ctx.enter_context(tc.tile_pool(name="hp", bufs=2))
        gp = ctx.enter_context(tc.tile_pool(name="gp", bufs=1))
        psU = ctx.enter_context(tc.tile_pool(name="psU", bufs=2, space="PSUM"))
        psA = ctx.enter_context(tc.tile_pool(name="psA", bufs=2, space="PSUM"))
        psT = ctx.enter_context(tc.tile_pool(name="psT", bufs=1, space="PSUM"))
        psS = ctx.enter_context(tc.tile_pool(name="psS", bufs=2, space="PSUM"))

        ident = consts.tile([128, 128], f32)
        make_identity(nc, ident[:])
        onesrow = consts.tile([1, 128], f32)
        nc.vector.memset(onesrow[:], 1.0)
        onescol = consts.tile([128, 1], f32)
        nc.vector.memset(onescol[:], 1.0)

        walls, brows = [], []
        for i, (fi, fo) in enumerate(LAYERS):
            wt = consts.tile([fi, 5 * fo], f32, tag=f"wall{i}")
            nc.sync.dma_start(out=wt[:], in_=wall_d[i])
            walls.append(wt)
            bt = consts.tile([1, 5 * fo], f32, tag=f"brow{i}")
            nc.sync.dma_start(out=bt[:], in_=brow_d[i])
            brows.append(bt)
        fcw1 = consts.tile([HID, NCLS], f32, tag="fcw1")
        nc.sync.dma_start(out=fcw1[:], in_=fcw1_d)
        fcb1 = consts.tile([1, NCLS], f32, tag="fcb1")
        nc.sync.dma_start(out=fcb1[:], in_=fcb1_d)
        fcw2 = consts.tile([NCLS, NCLS], f32, tag="fcw2")
        nc.sync.dma_start(out=fcw2[:], in_=fcw2_d)
        fcb2 = consts.tile([1, NCLS], f32, tag="fcb2")
        nc.sync.dma_start(out=fcb2[:], in_=fcb2_d)

        gbuf = gp.tile([HID, GPC], f32)

        for g in range(GPC):
            lt = ltp.tile([128, 8 * 1024], f32, tag="lt")
            nc.sync.dma_start(out=lt[:], in_=lt_d[g])
            xf = xtp.tile([128, 1024], f32, tag="xf")
            nc.sync.dma_start(out=xf[:], in_=xt_d[g])

            X = xf
            for li, (fi, fo) in enumerate(LAYERS):
                wall, brow = walls[li], brows[li]
                # --- projections U_k = X @ W_k (+ bias on k=0), node-layout
                U = up.tile([128, 8 * 5 * fo], f32, tag=f"U{li}")
                for m in range(8):
                    pu = psU.tile([128, 5 * fo], f32, tag="pu")
                    nc.tensor.matmul(
                        pu[:],
                        lhsT=X[:fi, m * 128 : (m + 1) * 128],
                        rhs=wall[:],
                        start=True,
                        stop=False,
                    )
                    nc.tensor.matmul(
                        pu[:], lhsT=onesrow[:1, :128], rhs=brow[:], start=False, stop=True
                    )
                    nc.scalar.copy(U[:, m * 5 * fo : (m + 1) * 5 * fo], pu[:])

                def Uk(k):
                    # strided [128, 8, fo] view: chebyshev block k of every node-chunk
                    return U[:].rearrange("p (m x) -> p m x", x=5 * fo)[
                        :, :, k * fo : (k + 1) * fo
                    ]

                def Ukchunk(m, k):
                    base = m * 5 * fo + k * fo
                    return U[:, base : base + fo]

                def r3(t):
                    return t[:].rearrange("p (m x) -> p m x", x=fo)

                def app(rhs_fn):
                    pa = psA.tile([128, 8 * fo], f32, tag="pa")
                    for m in range(8):
                        o = pa[:, m * fo : (m + 1) * fo]
                        for k in range(8):
                            nc.tensor.matmul(
                                o,
                                lhsT=lt[:, k * 1024 + m * 128 : k * 1024 + (m + 1) * 128],
                                rhs=rhs_fn(k),
                                start=(k == 0),
                                stop=(k == 7),
                            )
                    return pa

                b3 = bp.tile([128, 8 * fo], f32, tag="b3")
                b2 = bp.tile([128, 8 * fo], f32, tag="b2")
                b1 = bp.tile([128, 8 * fo], f32, tag="b1")
                D2 = bp.tile([128, 8 * fo], f32, tag="D2")
                D1 = bp.tile([128, 8 * fo], f32, tag="D1")
                D0 = bp.tile([128, 8 * fo], f32, tag="D0")
                hpre = hp.tile([128, 8 * fo], f32, tag="hpre")
                hout = hp.tile([128, 8 * fo], f32, tag="hout")

                # D2 = U2 - b4 (b4 == U4), independent of apps
                nc.vector.tensor_sub(r3(D2), Uk(2), Uk(4))

                # b3 = 2L b4 + U3
                pa = app(lambda k: Ukchunk(k, 4))
                nc.vector.tensor_add(r3(b3), pa[:].rearrange("p (m x) -> p m x", x=fo), Uk(3))
                nc.vector.tensor_sub(r3(D1), Uk(1), r3(b3))

                # b2 = 2L b3 + D2
                pa = app(lambda k: b3[:, k * fo : (k + 1) * fo])
                nc.vector.tensor_add(b2[:], pa[:], D2[:])
                nc.vector.tensor_sub(r3(D0), Uk(0), r3(b2))

                # b1 = 2L b2 + D1
                pa = app(lambda k: b2[:, k * fo : (k + 1) * fo])
                nc.vector.tensor_add(b1[:], pa[:], D1[:])

                # h = relu(0.5 * (2L b1) + D0)
                pa = app(lambda k: b1[:, k * fo : (k + 1) * fo])
                nc.vector.scalar_tensor_tensor(
                    hpre[:], pa[:], 0.5, D0[:], op0=Alu.mult, op1=Alu.add
                )
                nc.scalar.activation(hout[:], hpre[:], Act.Relu)

                if li < 2:
                    # transpose h -> feature-layout input of next layer
                    pt = psT.tile([fo, 1024], f32, tag="pt")
                    for c in range(8):
                        nc.tensor.transpose(
                            pt[:, c * 128 : (c + 1) * 128],
                            hout[:, c * fo : (c + 1) * fo],
                            ident[:],
                        )
                    Xn = xfp.tile([fo, 1024], f32, tag=f"X{li + 1}")
                    nc.scalar.copy(Xn[:], pt[:])
                    X = Xn
                else:
                    # global mean pool (1/1024 folded into fcw1 on host)
                    pp = psS.tile([HID, 1], f32, tag="small")
                    for k in range(8):
                        nc.tensor.matmul(
                            pp[:],
                            lhsT=hout[:, k * HID : (k + 1) * HID],
                            rhs=onescol[:],
                            start=(k == 0),
                            stop=(k == 7),
                        )
                    nc.scalar.copy(gbuf[:, g : g + 1], pp[:])

        # --- MLP head over all 8 graphs at once
        pm = psS.tile([GPC, NCLS], f32, tag="small")
        nc.tensor.matmul(pm[:], lhsT=gbuf[:], rhs=fcw1[:], start=True, stop=False)
        nc.tensor.matmul(
            pm[:], lhsT=onesrow[:1, :GPC], rhs=fcb1[:], start=False, stop=True
        )
        h1 = hp.tile([GPC, NCLS], f32, tag="mlph1")
        nc.scalar.activation(h1[:], pm[:], Act.Relu)

        ptm = psS.tile([NCLS, GPC], f32, tag="small")
        nc.tensor.transpose(ptm[:], h1[:], ident[:GPC, :GPC])
        h1t = hp.tile([NCLS, GPC], f32, tag="mlph1t")
        nc.scalar.copy(h1t[:], ptm[:])

        pf = psS.tile([GPC, NCLS], f32, tag="small")
        nc.tensor.matmul(pf[:], lhsT=h1t[:], rhs=fcw2[:], start=True, stop=False)
        nc.tensor.matmul(
            pf[:], lhsT=onesrow[:1, :GPC], rhs=fcb2[:], start=False, stop=True
        )
        ob = hp.tile([GPC, NCLS], f32, tag="ob")
        nc.vector.tensor_copy(ob[:], pf[:])
        nc.sync.dma_start(out=out_d, in_=ob[:])

    nc.compile()
    return nc


def _prep_inputs(x, edge_index, batch, lambda_max, W1, b1, W2, b2, W3, b3, fcW1, fcb1, fcW2, fcb2):
    x = np.asarray(x, np.float32)
    edge_index = np.asarray(edge_index, np.int64)
    batch = np.asarray(batch, np.int64)
    lambda_max = np.asarray(lambda_max, np.float32)

    src, dst = edge_index[0], edge_index[1]
    # the decomposition below requires block-aligned graphs; guaranteed by
    # the reference input generator
    assert np.array_equal(batch, np.arange(N_NODES) // NPG)
    assert ((src // NPG) == (dst // NPG)).all()

    mask = src != dst
    deg = np.bincount(src[mask], minlength=N_NODES).astype(np.float32)
    dis = np.where(deg > 0, 1.0 / np.sqrt(np.maximum(deg, 1.0)), 0.0).astype(np.float32)
    lam_e = lambda_max[batch[src]]
    w = np.where(mask, -2.0 * dis[src] * dis[dst] / lam_e, 0.0).astype(np.float32)
    diag = (2.0 / lambda_max[batch] - 1.0).astype(np.float32)

    ge = src // NPG
    sl = src % NPG
    dl = dst % NPG
    flat = (ge * NPG + sl) * NPG + dl
    M2 = np.bincount(flat, weights=(2.0 * w).astype(np.float64), minlength=G * NPG * NPG)
    M2 = M2.astype(np.float32).reshape(G, NPG, NPG)
    M2[:, np.arange(NPG), np.arange(NPG)] += 2.0 * diag.reshape(G, NPG)

    walls = []
    brows = []
    for Wl, bl in ((W1, b1), (W2, b2), (W3, b3)):
        Wl = np.asarray(Wl, np.float32)
        bl = np.asarray(bl, np.float32)
        fo = Wl.shape[2]
        walls.append(np.concatenate([Wl[k] for k in range(S)], axis=1))
        br = np.zeros((1, S * fo), np.float32)
        br[0, :fo] = bl
        brows.append(br)

    cnt = np.bincount(batch, minlength=G)
    assert (cnt == NPG).all()
    fcw1s = (np.asarray(fcW1, np.float32) / float(NPG)).astype(np.float32)

    in_maps = []
    for c in range(NCORES):
        gs = slice(c * GPC, (c + 1) * GPC)
        lt2 = (
            M2[gs]
            .reshape(GPC, 8, 128, NPG)
            .transpose(0, 2, 1, 3)
            .reshape(GPC, 128, 8 * NPG)
            .copy()
        )
        xt = (
            x[c * GPC * NPG : (c + 1) * GPC * NPG]
            .reshape(GPC, NPG, IN_F)
            .transpose(0, 2, 1)
            .copy()
        )
        m = {
            "lt2": lt2,
            "xt": xt,
            "fcw1": fcw1s,
            "fcb1": np.asarray(fcb1, np.float32).reshape(1, NCLS),
            "fcw2": np.asarray(fcW2, np.float32),
            "fcb2": np.asarray(fcb2, np.float32).reshape(1, NCLS),
        }
        for i in range(3):
            m[f"wall{i}"] = walls[i]
            m[f"brow{i}"] = brows[i]
        in_maps.append(m)
    return in_maps


def kernel(**inputs) -> np.ndarray:
    global LAST
    from concourse.bass_utils import run_bass_kernel_spmd

    in_maps = _prep_inputs(**inputs)
    if "nc" not in _CACHE:
        _CACHE["nc"] = _build_bass()
    nc = _CACHE["nc"]
    res = run_bass_kernel_spmd(
        nc,
        in_maps,
        list(range(NCORES)),
        trace=bool(os.environ.get("KERNEL_TRACE")),
    )
    LAST = res
    out = np.concatenate([res.results[c]["out"] for c in range(NCORES)], axis=0)
    return out.astype(np.float32)
